# revision 1
# baseline (speedup 1.0000x reference)
"""MEGADecoder forward pass as a Bass/Tile kernel on 8 TRN2 NeuronCores.

Sharding: sequence-parallel. Each core owns SEQ/8 = 512 rows. Params are
replicated. Single-head full attention uses one AllGather of (K.T, V).

Layout: activations are stored feature-major ([8 chunks x 128 partitions,
seq 512 free]) so every GEMM is a chain of 128x128x512 PE matmuls with no
transposes anywhere:
  - projections:  out.T[o, s] = sum_d W.T[d, o] . act.T[d, s]
  - V projection: V[s, o]     = sum_d Z.T[d, s] . Wv.T[d, o]   (seq-major out)
  - scores.T:     S.T[j, i]   = sum_f K.T[f, j] . Q.T[f, i]
  - attention:    Zat.T[o, s] = sum_j V[j, o]   . P.T[j, s]
Softmax runs without max-subtraction (scores for this model are O(1)) and the
denominator comes from a ones-vector matmul accumulated across j-chunks.
Matmul operands are float32r (TF32-like fast fp32 mode, 4x fp32 throughput).
"""

import numpy as np

SEQ = 4096
D = 1024
NCORES = 8
S = SEQ // NCORES  # 512 rows per core
P = 128
FC = D // P  # 8 feature chunks
ATT_SCALE = 1.0 / float(np.sqrt(np.float32(D)))

USE_F32R = True

_CACHE = {}


def _build_bass():
    import concourse.bacc as bacc
    import concourse.tile as tile
    import concourse.mybir as mybir

    f32 = mybir.dt.float32
    fmm = mybir.dt.float32r if USE_F32R else f32
    AF = mybir.ActivationFunctionType

    nc = bacc.Bacc(None, target_bir_lowering=False, num_devices=NCORES)
    mm = nc.tensor.matmul

    # ---- DRAM I/O ----
    rt = nc.dram_tensor("rt", [D, S + 1], fmm, kind="ExternalInput")
    w_in = {}
    for name, shp in [
        ("wa", [2 * D, D]), ("wd", [2 * D, D]), ("wz", [D, D]),
        ("wq", [D, D]), ("wk", [D, D]), ("wv", [D, D]),
        ("wema", [D, D]), ("wf", [D, D]), ("wzat", [D, D]),
        ("wi", [D, 1]), ("wfin", [D, 1]),
    ]:
        w_in[name] = nc.dram_tensor(name, shp, fmm, kind="ExternalInput")
    # biases packed [10, D]: rows alpha,delta,z,q(pre-scaled),k,v,ema,f,zat,i
    biases = nc.dram_tensor("biases", [10, D], f32, kind="ExternalInput")
    out = nc.dram_tensor("out", [S, 1], f32, kind="ExternalOutput")

    KT_ELEMS = D * S
    V_ELEMS = S * D
    NJ = SEQ // P  # 32 j-chunks

    with tile.TileContext(nc) as tc, \
         tc.tile_pool(name="consts", bufs=1) as consts, \
         tc.tile_pool(name="dram", bufs=1, space="DRAM") as dram, \
         tc.tile_pool(name="big", bufs=1) as big:

        row_bounce = dram.tile([2, S], f32)
        kv_in = dram.tile([KT_ELEMS + V_ELEMS], fmm)
        kv_out = dram.tile([NCORES, KT_ELEMS + V_ELEMS], fmm, addr_space="Shared")

        bsb = consts.tile([P, 10, FC], f32)
        nc.sync.dma_start(out=bsb, in_=biases.ap().rearrange("b (c p) -> p b c", p=P))
        ones_f32 = consts.tile([P, 1], f32)
        nc.vector.memset(ones_f32, 1.0)
        ones_col = consts.tile([P, 1], fmm)
        nc.scalar.copy(ones_col, ones_f32)

        def bias_ap(row, chunk):
            return bsb[:, row, chunk:chunk + 1]

        # whole-kernel resident activations (98KB/partition total)
        rema = big.tile([P, FC, S], fmm, name="rema")
        z = big.tile([P, FC, S], fmm, name="z")
        qT = big.tile([P, FC, S], fmm, name="qT")
        remap = big.tile([P, FC, S], fmm, name="remap")
        fT = big.tile([P, FC, S], f32, name="fT")
        zatp = big.tile([P, FC, S], fmm, name="zatp")
        ib = big.tile([P, S], f32, name="ib")

        # ---------------- Phase 1: R_EMA ----------------
        with tc.tile_pool(name="p_rt", bufs=1) as p_rt, \
             tc.tile_pool(name="p1w", bufs=3) as p1w, \
             tc.tile_pool(name="p1ps", bufs=2, space="PSUM") as p1ps, \
             tc.tile_pool(name="p1t", bufs=2) as p1t:
            rt_sb = p_rt.tile([P, FC, S + 1], fmm)
            nc.sync.dma_start(out=rt_sb, in_=rt.ap().rearrange("(c p) s -> p c s", p=P))
            for ot in range(FC):
                ow = slice(ot * P, (ot + 1) * P)
                wa_t = p1w.tile([P, 2 * FC, P], fmm, tag="wa")
                nc.sync.dma_start(
                    out=wa_t,
                    in_=w_in["wa"].ap()[:, ow].rearrange("(c p) o -> p c o", p=P))
                wd_t = p1w.tile([P, 2 * FC, P], fmm, tag="wd")
                nc.sync.dma_start(
                    out=wd_t,
                    in_=w_in["wd"].ap()[:, ow].rearrange("(c p) o -> p c o", p=P))
                ps_a = p1ps.tile([P, S], f32, tag="psa")
                ps_d = p1ps.tile([P, S], f32, tag="psd")
                for ch in range(FC):
                    mm(ps_a, wa_t[:, ch, :], rt_sb[:, ch, 0:S],
                       start=(ch == 0), stop=False)
                    mm(ps_d, wd_t[:, ch, :], rt_sb[:, ch, 0:S],
                       start=(ch == 0), stop=False)
                for ch in range(FC):
                    mm(ps_a, wa_t[:, FC + ch, :], rt_sb[:, ch, 1:S + 1],
                       start=False, stop=(ch == FC - 1))
                    mm(ps_d, wd_t[:, FC + ch, :], rt_sb[:, ch, 1:S + 1],
                       start=False, stop=(ch == FC - 1))
                alpha_t = p1t.tile([P, S], f32, tag="alpha")
                nc.scalar.activation(alpha_t, ps_a, AF.Tanh,
                                     bias=bias_ap(0, ot), scale=1.0)
                delta_t = p1t.tile([P, S], f32, tag="delta")
                nc.scalar.activation(delta_t, ps_d, AF.Tanh,
                                     bias=bias_ap(1, ot), scale=1.0)
                # rema = t1 + alpha*(r_t - t1), t1 = delta*r_prev
                t1 = p1t.tile([P, S], f32, tag="t1")
                nc.vector.tensor_mul(t1, delta_t, rt_sb[:, ot, 0:S])
                t2 = p1t.tile([P, S], f32, tag="t2")
                nc.vector.tensor_sub(t2, rt_sb[:, ot, 1:S + 1], t1)
                t3 = p1t.tile([P, S], f32, tag="t3")
                nc.vector.tensor_mul(t3, alpha_t, t2)
                nc.vector.tensor_add(rema[:, ot, :], t3, t1)

        # ---------------- Phase 2+3: Z, Q.T, K.T, V + AllGather ----------------
        with tc.tile_pool(name="p2w", bufs=3) as p2w, \
             tc.tile_pool(name="p2ps", bufs=4, space="PSUM") as p2ps, \
             tc.tile_pool(name="p_kv", bufs=1) as p_kv:
            def proj(w_name, rhs_src, out_tile, func, bias_row, scale=1.0):
                for half in range(2):
                    osl = slice(half * 4 * P, (half + 1) * 4 * P)
                    w_t = p2w.tile([P, FC, 4 * P], fmm, tag="w")
                    nc.sync.dma_start(
                        out=w_t,
                        in_=w_in[w_name].ap()[:, osl].rearrange("(c p) o -> p c o", p=P))
                    for sub in range(4):
                        ot = half * 4 + sub
                        ow = slice(sub * P, (sub + 1) * P)
                        ps = p2ps.tile([P, S], f32, tag="ps")
                        for ch in range(FC):
                            mm(ps, w_t[:, ch, ow], rhs_src[:, ch, :],
                               start=(ch == 0), stop=(ch == FC - 1))
                        nc.scalar.activation(out_tile[:, ot, :], ps, func,
                                             bias=bias_ap(bias_row, ot), scale=scale)

            proj("wz", rema, z, AF.Silu, 2)
            proj("wq", z, qT, AF.Identity, 3, scale=ATT_SCALE)

            ktS = p_kv.tile([P, FC, S], fmm)
            proj("wk", z, ktS, AF.Identity, 4)
            nc.sync.dma_start(
                out=kv_in[0:KT_ELEMS].rearrange("(c p s) -> p c s", p=P, s=S),
                in_=ktS)

            # V seq-major: V[s, o] = sum_d Z.T[d, s] Wv.T[d, o] (+ bv)
            bv_b = p_kv.tile([P, D], f32, tag="bvb")
            nc.sync.dma_start(
                out=bv_b, in_=biases.ap()[5:6, :].partition_broadcast(P))
            for half in range(2):
                osl = slice(half * 4 * P, (half + 1) * 4 * P)
                wv_t = p2w.tile([P, FC, 4 * P], fmm, tag="w")
                nc.sync.dma_start(
                    out=wv_t,
                    in_=w_in["wv"].ap()[:, osl].rearrange("(c p) o -> p c o", p=P))
                for st in range(4):
                    ssl = slice(st * P, (st + 1) * P)
                    ps = p2ps.tile([P, 4 * P], f32, tag="ps")
                    for ch in range(FC):
                        mm(ps, z[:, ch, ssl], wv_t[:, ch, :],
                           start=(ch == 0), stop=(ch == FC - 1))
                    v_sb = p_kv.tile([P, 4 * P], fmm, tag="vsb")
                    nc.vector.tensor_add(v_sb, ps, bv_b[:, osl])
                    nc.sync.dma_start(
                        out=kv_in[KT_ELEMS + st * P * D:
                                  KT_ELEMS + (st + 1) * P * D].rearrange(
                            "(p o) -> p o", p=P)[:, osl],
                        in_=v_sb)

            nc.gpsimd.collective_compute(
                "AllGather", mybir.AluOpType.bypass,
                replica_groups=[list(range(NCORES))],
                ins=[kv_in[:].opt()], outs=[kv_out[:].opt()],
            )

        # ---------------- Phase 4: R_EMA', f, i ----------------
        with tc.tile_pool(name="p4w", bufs=3) as p4w, \
             tc.tile_pool(name="p4ps", bufs=2, space="PSUM") as p4ps, \
             tc.tile_pool(name="p4psi", bufs=1, space="PSUM") as p4psi, \
             tc.tile_pool(name="p4t", bufs=1) as p4t:
            def proj4(w_name, rhs_src, out_tile, func, bias_row):
                for half in range(2):
                    osl = slice(half * 4 * P, (half + 1) * 4 * P)
                    w_t = p4w.tile([P, FC, 4 * P], fmm, tag="w")
                    nc.sync.dma_start(
                        out=w_t,
                        in_=w_in[w_name].ap()[:, osl].rearrange("(c p) o -> p c o", p=P))
                    for sub in range(4):
                        ot = half * 4 + sub
                        ow = slice(sub * P, (sub + 1) * P)
                        ps = p4ps.tile([P, S], f32, tag="ps")
                        for ch in range(FC):
                            mm(ps, w_t[:, ch, ow], rhs_src[:, ch, :],
                               start=(ch == 0), stop=(ch == FC - 1))
                        nc.scalar.activation(out_tile[:, ot, :], ps, func,
                                             bias=bias_ap(bias_row, ot), scale=1.0)

            proj4("wema", rema, remap, AF.Identity, 6)
            proj4("wf", remap, fT, AF.Sigmoid, 7)

            wi_sb = p4t.tile([P, FC], fmm, tag="wi")
            nc.sync.dma_start(
                out=wi_sb, in_=w_in["wi"].ap().rearrange("(c p) o -> p (c o)", p=P))
            ps_i = p4psi.tile([1, S], f32)
            for ch in range(FC):
                mm(ps_i, wi_sb[:, ch:ch + 1], rema[:, ch, :],
                   start=(ch == 0), stop=(ch == FC - 1))
            i_row = p4t.tile([1, S], f32, tag="irow")
            nc.scalar.activation(i_row, ps_i, AF.Tanh,
                                 bias=bsb[0:1, 9, 0:1], scale=1.0)
            nc.sync.dma_start(out=row_bounce[0:1, :], in_=i_row)
            nc.sync.dma_start(out=ib, in_=row_bounce[0:1, :].partition_broadcast(P))

        # ---------------- Phase 5: attention ----------------
        with tc.tile_pool(name="p_pt", bufs=1) as p_pt, \
             tc.tile_pool(name="p_rl", bufs=1) as p_rl:
            pt = p_pt.tile([P, NJ, S], fmm)
            rl_b = p_rl.tile([P, S], f32, tag="rlb")
            # --- 5A: scores.T + exp + denominator ---
            with tc.tile_pool(name="p5w", bufs=6) as p5w, \
                 tc.tile_pool(name="p5ps", bufs=2, space="PSUM") as p5ps, \
                 tc.tile_pool(name="p5lps", bufs=1, space="PSUM") as p5lps:
                l_ps = p5lps.tile([1, S], f32)
                for jc in range(NJ):
                    r, cl = jc // 4, jc % 4
                    kt_ch = p5w.tile([P, FC, P], fmm, tag="ktch")
                    nc.sync.dma_start(
                        out=kt_ch,
                        in_=kv_out[r, 0:KT_ELEMS].rearrange(
                            "(c p s) -> p c s", p=P, s=S)[:, :, cl * P:(cl + 1) * P])
                    s_ps = p5ps.tile([P, S], f32, tag="sps")
                    for ch in range(FC):
                        mm(s_ps, kt_ch[:, ch, :], qT[:, ch, :],
                           start=(ch == 0), stop=(ch == FC - 1))
                    nc.scalar.activation(pt[:, jc, :], s_ps, AF.Exp,
                                         bias=0.0, scale=1.0)
                    mm(l_ps, ones_col, pt[:, jc, :],
                       start=(jc == 0), stop=(jc == NJ - 1))
                l_row = p_rl.tile([1, S], f32, tag="lrow")
                nc.vector.reciprocal(l_row, l_ps)
                nc.sync.dma_start(out=row_bounce[1:2, :], in_=l_row)
                nc.sync.dma_start(out=rl_b, in_=row_bounce[1:2, :].partition_broadcast(P))

            # --- 5B: Zat.T ---
            with tc.tile_pool(name="p5v", bufs=6) as p5v, \
                 tc.tile_pool(name="pvps", bufs=1, space="PSUM") as pvps:
                zat_ps = []
                for i in range(FC):
                    zp = pvps.tile([P, S], f32, tag=f"zat{i}", name=f"zat{i}")
                    zat_ps.append(zp)
                for jc in range(NJ):
                    r, cl = jc // 4, jc % 4
                    v_ch = p5v.tile([P, D], fmm, tag="vch")
                    nc.sync.dma_start(
                        out=v_ch,
                        in_=kv_out[r, KT_ELEMS:].rearrange(
                            "(t p o) -> t p o", p=P, o=D)[cl])
                    for ot in range(FC):
                        mm(zat_ps[ot], v_ch[:, ot * P:(ot + 1) * P], pt[:, jc, :],
                           start=(jc == 0), stop=(jc == NJ - 1))
                # zatp = f * (zat/l)
                for ot in range(FC):
                    nc.vector.tensor_mul(zat_ps[ot], zat_ps[ot], rl_b)
                    nc.vector.tensor_mul(zatp[:, ot, :], zat_ps[ot], fT[:, ot, :])

        # ---------------- Phase 6: output head ----------------
        with tc.tile_pool(name="p6w", bufs=2) as p6w, \
             tc.tile_pool(name="p6ps", bufs=2, space="PSUM") as p6ps, \
             tc.tile_pool(name="p6t", bufs=2) as p6t, \
             tc.tile_pool(name="p6fps", bufs=1, space="PSUM") as p6fps:
            wfin_sb = p6t.tile([P, FC], fmm, tag="wfin")
            nc.sync.dma_start(
                out=wfin_sb, in_=w_in["wfin"].ap().rearrange("(c p) o -> p (c o)", p=P))
            fin_ps = p6fps.tile([1, S], f32)
            for half in range(2):
                osl = slice(half * 4 * P, (half + 1) * 4 * P)
                w_t = p6w.tile([P, FC, 4 * P], fmm, tag="w")
                nc.sync.dma_start(
                    out=w_t,
                    in_=w_in["wzat"].ap()[:, osl].rearrange("(c p) o -> p c o", p=P))
                for sub in range(4):
                    ot = half * 4 + sub
                    ow = slice(sub * P, (sub + 1) * P)
                    ps = p6ps.tile([P, S], f32, tag="ps")
                    for ch in range(FC):
                        mm(ps, w_t[:, ch, ow], zatp[:, ch, :],
                           start=(ch == 0), stop=(ch == FC - 1))
                    t_sum = p6t.tile([P, S], f32, tag="tsum")
                    nc.vector.tensor_add(t_sum, ps, remap[:, ot, :])
                    ztp = p6t.tile([P, S], f32, tag="ztp")
                    nc.scalar.activation(ztp, t_sum, AF.Tanh,
                                         bias=bias_ap(8, ot), scale=1.0)
                    # zf = remap + ib*(ztp - remap)
                    d_t = p6t.tile([P, S], f32, tag="dt")
                    nc.vector.tensor_sub(d_t, ztp, remap[:, ot, :])
                    m_t = p6t.tile([P, S], f32, tag="mt")
                    nc.vector.tensor_mul(m_t, d_t, ib)
                    zf = p6t.tile([P, S], fmm, tag="zf")
                    nc.vector.tensor_add(zf, m_t, remap[:, ot, :])
                    mm(fin_ps, wfin_sb[:, ot:ot + 1], zf,
                       start=(ot == 0), stop=(ot == FC - 1))
            phat = p6t.tile([1, S], f32, tag="phat")
            nc.scalar.activation(phat, fin_ps, AF.Sigmoid, bias=0.0, scale=1.0)
            nc.sync.dma_start(out=out.ap().rearrange("s o -> o s"), in_=phat)
    nc.finalize()
    return nc


def _prep_host_inputs(inputs):
    """Transpose weights / build per-core shards (pure layout work)."""
    R = np.ascontiguousarray(inputs["R"], dtype=np.float32)
    RT_ext = np.concatenate(
        [np.zeros((D, 1), np.float32), np.ascontiguousarray(R.T)], axis=1)

    w = {
        "wa": inputs["W_alpha"].T, "wd": inputs["W_delta"].T,
        "wz": inputs["W_z"].T, "wq": inputs["W_q"].T, "wk": inputs["W_k"].T,
        "wv": inputs["W_v"].T, "wema": inputs["W_EMA"].T, "wf": inputs["W_f"].T,
        "wzat": inputs["W_z_at"].T, "wi": inputs["W_i"].T,
        "wfin": inputs["W_final"].T,
    }
    w = {k: np.ascontiguousarray(v, dtype=np.float32) for k, v in w.items()}

    biases = np.zeros((10, D), np.float32)
    biases[0] = inputs["b_alpha"]
    biases[1] = inputs["b_delta"]
    biases[2] = inputs["b_z"]
    biases[3] = inputs["b_q"] * ATT_SCALE
    biases[4] = inputs["b_k"]
    biases[5] = inputs["b_v"]
    biases[6] = inputs["b_EMA"]
    biases[7] = inputs["b_f"]
    biases[8] = inputs["b_z_at"]
    biases[9, 0] = np.float32(inputs["b_i"][0])

    in_maps = []
    for c in range(NCORES):
        m = {"rt": np.ascontiguousarray(RT_ext[:, c * S:c * S + S + 1]),
             "biases": biases}
        m.update(w)
        in_maps.append(m)
    return in_maps


def kernel(**inputs):
    from concourse.bass_utils import run_bass_kernel_spmd

    if "nc" not in _CACHE:
        _CACHE["nc"] = _build_bass()
    nc = _CACHE["nc"]
    in_maps = _prep_host_inputs(inputs)
    res = run_bass_kernel_spmd(nc, in_maps, core_ids=list(range(NCORES)))
    outs = [res.results[c]["out"] for c in range(NCORES)]
    return np.concatenate(outs, axis=0).astype(np.float32)



# revision 57
# speedup vs baseline: 179.6130x; 179.6130x over previous
"""MEGADecoder forward pass as a Bass/Tile kernel on 8 TRN2 NeuronCores.

Sharding: sequence-parallel. Each core owns SEQ/8 = 512 rows. Params are
replicated. Single-head full attention uses one AllGather of (K.T, V).

Layout: activations are stored feature-major ([8 chunks x 128 partitions,
seq 512 free]) so every GEMM is a chain of 128x128x512 PE matmuls with no
transposes anywhere:
  - projections:  out.T[o, s] = sum_d W.T[d, o] . act.T[d, s]
  - V projection: V[s, o]     = sum_d Z.T[d, s] . Wv.T[d, o]   (seq-major out)
  - scores.T:     S.T[j, i]   = sum_f K.T[f, j] . Q.T[f, i]
  - attention:    Zat.T[o, s] = sum_j V[j, o]   . P.T[j, s]
Softmax runs without max-subtraction (scores for this model are O(1)) and the
denominator comes from a ones-vector matmul accumulated across j-chunks.

Optimizations vs the fp32r baseline (521us -> ~351us device time/iter):
  - all matmul operands are bf16 (PSUM accumulation stays fp32): halves HBM
    traffic and the AllGather payload; PE rate is unchanged but weight loads
    pipeline fully (measured 171-175 ns per 128x128x512 matmul in chains).
  - every weight / input is host-packed into the exact SBUF layout, so each
    DMA reads one contiguous >=512B line per partition. DMA granularity is
    ~0.5-1MB: one giant DMA runs on too few SDMA engines and serializes its
    consumers (4MB whole-weight loads regressed 370->408us), while many tiny
    DMAs pay ~0.4us trigger overhead each (0.25MB quarters regressed
    351->390us).
  - K.T is gathered the moment it lands in DRAM, V right after; the Q / EMA
    / f / i projections are emitted after the collectives and hide them
    (ablation: removing both collectives saves only ~9us).
  - attention loads K.T / V per remote rank as two parallel 0.5MB DMAs,
    triple-buffered across the rank loop (two in flight beat one 1MB DMA by
    ~20us over the kernel: per-DMA engine parallelism is limited).
  - softmax-denominator matmuls run after all score chains, and the output
    head runs all 8 W_zat chains before the 8 W_final matmuls — otherwise
    each in-order PE queue entry stalls on a pending exp/tanh from ACT.
  - the output head is algebraically split: Zfinal@Wf = i*(Ztp@Wf) +
    (1-i)*(remap@Wf); the remap branch is computed in phase 4 (hidden), and
    1/l is folded into the attention gate f during 5B's matmuls, so the
    post-attention tail is 8 DVE muls + a [1,512] blend.
"""

import numpy as np

SEQ = 4096
D = 1024
NCORES = 8
S = SEQ // NCORES  # 512 rows per core
P = 128
FC = D // P  # 8 feature chunks
ATT_SCALE = 1.0 / float(np.sqrt(np.float32(D)))

_CACHE = {}


def _build_bass(reps=1, nocoll=False, noatt=False):
    import concourse.bacc as bacc
    import concourse.tile as tile
    import concourse.mybir as mybir

    f32 = mybir.dt.float32
    bf16 = mybir.dt.bfloat16
    AF = mybir.ActivationFunctionType

    nc = bacc.Bacc(None, target_bir_lowering=False, num_devices=NCORES)
    mm = nc.tensor.matmul

    # ---- DRAM I/O (all host-packed layouts) ----
    rt = nc.dram_tensor("rt", [P, FC, S + 1], bf16, kind="ExternalInput")
    # phase-1 weights: [ot, p, k(2FC), o(P)]
    wa = nc.dram_tensor("wa", [FC, P, 2 * FC, P], bf16, kind="ExternalInput")
    wd = nc.dram_tensor("wd", [FC, P, 2 * FC, P], bf16, kind="ExternalInput")
    # projection weights: [half, p, c(FC), o(4P)] — one ~1MB DMA per half;
    # mid-size DMAs beat whole-weight loads (single-DMA bandwidth is limited)
    w_in = {}
    for name in ["wz", "wq", "wk", "wv", "wema", "wf", "wzat"]:
        w_in[name] = nc.dram_tensor(name, [2, P, FC, 4 * P], bf16,
                                    kind="ExternalInput")
    wi = nc.dram_tensor("wi", [P, FC], bf16, kind="ExternalInput")
    wfin = nc.dram_tensor("wfin", [P, FC], bf16, kind="ExternalInput")
    # biases packed [p, 10, FC]: rows alpha,delta,z,q(pre-scaled),k,v,ema,f,zat,i
    biases = nc.dram_tensor("biases", [P, 10, FC], f32, kind="ExternalInput")
    bvrow = nc.dram_tensor("bvrow", [1, D], f32, kind="ExternalInput")
    out = nc.dram_tensor("out", [S, 1], f32, kind="ExternalOutput")

    KT_ELEMS = D * S
    V_ELEMS = S * D
    NJ = SEQ // P  # 32 j-chunks

    with tile.TileContext(nc) as tc, \
         tc.tile_pool(name="consts", bufs=1) as consts, \
         tc.tile_pool(name="dram", bufs=1, space="DRAM") as dram, \
         tc.tile_pool(name="big", bufs=1) as big:

        bsb = consts.tile([P, 10, FC], f32)
        nc.sync.dma_start(out=bsb, in_=biases.ap())
        ones_f32 = consts.tile([P, 1], f32)
        nc.vector.memset(ones_f32, 1.0)
        ones_col = consts.tile([P, 1], bf16)
        nc.scalar.copy(ones_col, ones_f32)
        bv_b = consts.tile([P, D], f32)
        nc.sync.dma_start(out=bv_b, in_=bvrow.ap().partition_broadcast(P))

        def bias_ap(row, chunk):
            return bsb[:, row, chunk:chunk + 1]

        # whole-kernel resident activations (bf16: ~66KB/partition total)
        rema = big.tile([P, FC, S], bf16, name="rema")
        z = big.tile([P, FC, S], bf16, name="z")
        qT = big.tile([P, FC, S], bf16, name="qT")
        remap = big.tile([P, FC, S], bf16, name="remap")
        fT = big.tile([P, FC, S], f32, name="fT")
        zatp = big.tile([P, FC, S], bf16, name="zatp")
        i_row = big.tile([1, S], f32, name="i_row")
        fin_rem = big.tile([1, S], f32, name="fin_rem")

        for _rep in range(reps):
            row_bounce = dram.tile([1, S], f32, name=f"rb{_rep}")
            kt_in = dram.tile([KT_ELEMS], bf16, name=f"kti{_rep}")
            v_in = dram.tile([V_ELEMS], bf16, name=f"vi{_rep}")
            if nocoll:
                kt_out = v_out = None
            else:
                kt_out = dram.tile([NCORES, KT_ELEMS], bf16,
                                   addr_space="Shared", name=f"kto{_rep}")
                v_out = dram.tile([NCORES, V_ELEMS], bf16,
                                  addr_space="Shared", name=f"vo{_rep}")
            _emit_body(nc, tc, mybir, AF, bf16, f32, mm, rt, wa, wd, w_in,
                       wi, wfin, out, row_bounce, kt_in, kt_out, v_in, v_out,
                       bsb, bias_ap, ones_col, bv_b, rema, z, qT, remap, fT,
                       zatp, i_row, fin_rem, KT_ELEMS, V_ELEMS, NJ,
                       nocoll, noatt)
    nc.finalize()
    return nc


def _emit_body(nc, tc, mybir, AF, bf16, f32, mm, rt, wa, wd, w_in, wi, wfin,
               out, row_bounce, kt_in, kt_out, v_in, v_out, bsb, bias_ap,
               ones_col, bv_b, rema, z, qT, remap, fT, zatp, i_row, fin_rem,
               KT_ELEMS, V_ELEMS, NJ, nocoll=False, noatt=False):
    # ---------------- Phase 1: R_EMA ----------------
    with tc.tile_pool(name="p_rt", bufs=1) as p_rt, \
         tc.tile_pool(name="p1w", bufs=3) as p1w, \
         tc.tile_pool(name="p1ps", bufs=3, space="PSUM") as p1ps, \
         tc.tile_pool(name="p1t", bufs=2) as p1t:
        rt_sb = p_rt.tile([P, FC, S + 1], bf16)
        nc.sync.dma_start(out=rt_sb, in_=rt.ap())
        for ot in range(FC):
            wa_t = p1w.tile([P, 2 * FC, P], bf16, tag="wa")
            nc.sync.dma_start(out=wa_t, in_=wa.ap()[ot])
            wd_t = p1w.tile([P, 2 * FC, P], bf16, tag="wd")
            nc.sync.dma_start(out=wd_t, in_=wd.ap()[ot])
            ps_a = p1ps.tile([P, S], f32, tag="psa")
            ps_d = p1ps.tile([P, S], f32, tag="psd")
            for ch in range(FC):
                mm(ps_a, wa_t[:, ch, :], rt_sb[:, ch, 0:S],
                   start=(ch == 0), stop=False)
                mm(ps_d, wd_t[:, ch, :], rt_sb[:, ch, 0:S],
                   start=(ch == 0), stop=False)
            for ch in range(FC):
                mm(ps_a, wa_t[:, FC + ch, :], rt_sb[:, ch, 1:S + 1],
                   start=False, stop=(ch == FC - 1))
                mm(ps_d, wd_t[:, FC + ch, :], rt_sb[:, ch, 1:S + 1],
                   start=False, stop=(ch == FC - 1))
            alpha_t = p1t.tile([P, S], f32, tag="alpha")
            nc.scalar.activation(alpha_t, ps_a, AF.Tanh,
                                 bias=bias_ap(0, ot), scale=1.0)
            delta_t = p1t.tile([P, S], f32, tag="delta")
            nc.scalar.activation(delta_t, ps_d, AF.Tanh,
                                 bias=bias_ap(1, ot), scale=1.0)
            # rema = t1 + alpha*(r_t - t1), t1 = delta*r_prev
            t1 = p1t.tile([P, S], f32, tag="t1")
            nc.vector.tensor_mul(t1, delta_t, rt_sb[:, ot, 0:S])
            t2 = p1t.tile([P, S], f32, tag="t2")
            nc.vector.tensor_sub(t2, rt_sb[:, ot, 1:S + 1], t1)
            t3 = p1t.tile([P, S], f32, tag="t3")
            nc.vector.tensor_mul(t3, alpha_t, t2)
            nc.vector.tensor_add(rema[:, ot, :], t3, t1)

    # ---------------- Phase 2: Z, K.T, V + AllGather; then Q ----------------
    with tc.tile_pool(name="p2w", bufs=3) as p2w, \
         tc.tile_pool(name="p2ps", bufs=4, space="PSUM") as p2ps, \
         tc.tile_pool(name="p_kv", bufs=1) as p_kv:
        def proj(w_name, rhs_src, out_tile, func, bias_row, scale=1.0):
            for half in range(2):
                w_t = p2w.tile([P, FC, 4 * P], bf16, tag="w")
                nc.sync.dma_start(out=w_t, in_=w_in[w_name].ap()[half])
                for sub in range(4):
                    ot = half * 4 + sub
                    ow = slice(sub * P, (sub + 1) * P)
                    ps = p2ps.tile([P, S], f32, tag="ps")
                    for ch in range(FC):
                        mm(ps, w_t[:, ch, ow], rhs_src[:, ch, :],
                           start=(ch == 0), stop=(ch == FC - 1))
                    nc.scalar.activation(out_tile[:, ot, :], ps, func,
                                         bias=bias_ap(bias_row, ot), scale=scale)

        proj("wz", rema, z, AF.Silu, 2)

        # K.T -> kt_in (feature-major, partition-contiguous), gather ASAP
        ktS = p_kv.tile([P, FC, S], bf16)
        proj("wk", z, ktS, AF.Identity, 4)
        nc.sync.dma_start(
            out=kt_in[:].rearrange("(p c s) -> p c s", p=P, s=S),
            in_=ktS)
        if not nocoll:
            nc.gpsimd.collective_compute(
                "AllGather", mybir.AluOpType.bypass,
                replica_groups=[list(range(NCORES))],
                ins=[kt_in[:].opt()], outs=[kt_out[:].opt()],
            )

        # V seq-major: V[s, o] = sum_d Z.T[d, s] Wv.T[d, o] (+ bv)
        for half in range(2):
            osl = slice(half * 4 * P, (half + 1) * 4 * P)
            wv_t = p2w.tile([P, FC, 4 * P], bf16, tag="w")
            nc.sync.dma_start(out=wv_t, in_=w_in["wv"].ap()[half])
            for st in range(4):
                ssl = slice(st * P, (st + 1) * P)
                ps = p2ps.tile([P, 4 * P], f32, tag="ps")
                for ch in range(FC):
                    mm(ps, z[:, ch, ssl], wv_t[:, ch, :],
                       start=(ch == 0), stop=(ch == FC - 1))
                v_sb = p_kv.tile([P, 4 * P], bf16, tag="vsb")
                nc.vector.tensor_add(v_sb, ps, bv_b[:, osl])
                nc.sync.dma_start(
                    out=v_in[st * P * D:(st + 1) * P * D].rearrange(
                        "(p o) -> p o", p=P)[:, osl],
                    in_=v_sb)
        if not nocoll:
            nc.gpsimd.collective_compute(
                "AllGather", mybir.AluOpType.bypass,
                replica_groups=[list(range(NCORES))],
                ins=[v_in[:].opt()], outs=[v_out[:].opt()],
            )

        # Q (overlaps the AllGathers)
        proj("wq", z, qT, AF.Identity, 3, scale=ATT_SCALE)

    # ---------------- Phase 4: R_EMA', f, i (overlap AllGather) ----------------
    with tc.tile_pool(name="p4w", bufs=3) as p4w, \
         tc.tile_pool(name="p4ps", bufs=2, space="PSUM") as p4ps, \
         tc.tile_pool(name="p4psi", bufs=2, space="PSUM") as p4psi, \
         tc.tile_pool(name="p4t", bufs=1) as p4t:
        def proj4(w_name, rhs_src, out_tile, func, bias_row):
            for half in range(2):
                w_t = p4w.tile([P, FC, 4 * P], bf16, tag="w")
                nc.sync.dma_start(out=w_t, in_=w_in[w_name].ap()[half])
                for sub in range(4):
                    ot = half * 4 + sub
                    ow = slice(sub * P, (sub + 1) * P)
                    ps = p4ps.tile([P, S], f32, tag="ps")
                    for ch in range(FC):
                        mm(ps, w_t[:, ch, ow], rhs_src[:, ch, :],
                           start=(ch == 0), stop=(ch == FC - 1))
                    nc.scalar.activation(out_tile[:, ot, :], ps, func,
                                         bias=bias_ap(bias_row, ot), scale=1.0)

        proj4("wema", rema, remap, AF.Identity, 6)
        proj4("wf", remap, fT, AF.Sigmoid, 7)

        wi_sb = p4t.tile([P, FC], bf16, tag="wi")
        nc.sync.dma_start(out=wi_sb, in_=wi.ap())
        ps_i = p4psi.tile([1, S], f32)
        for ch in range(FC):
            mm(ps_i, wi_sb[:, ch:ch + 1], rema[:, ch, :],
               start=(ch == 0), stop=(ch == FC - 1))
        nc.scalar.activation(i_row, ps_i, AF.Tanh,
                             bias=bsb[0:1, 9, 0:1], scale=1.0)

        # fin_rem = remap @ W_final.T  (the (1-i) branch of the output head)
        wfin_sb = p4t.tile([P, FC], bf16, tag="wfin")
        nc.sync.dma_start(out=wfin_sb, in_=wfin.ap())
        ps_fr = p4psi.tile([1, S], f32, tag="psfr")
        for ch in range(FC):
            mm(ps_fr, wfin_sb[:, ch:ch + 1], remap[:, ch, :],
               start=(ch == 0), stop=(ch == FC - 1))
        nc.scalar.copy(fin_rem, ps_fr)

    # ---------------- Phase 5: attention ----------------
    if noatt:
        nc.vector.memset(zatp, 0.001)
        nc.vector.memset(fT[:, 0, :], 1.0)
        _emit_head(nc, tc, AF, bf16, f32, mm, w_in, wfin, out, bsb, bias_ap,
                   remap, zatp, i_row, fin_rem)
        return
    with tc.tile_pool(name="p_pt", bufs=1) as p_pt, \
         tc.tile_pool(name="p_rl", bufs=1) as p_rl:
        pt = p_pt.tile([P, NJ, S], bf16)
        rl_b = p_rl.tile([P, S], f32, tag="rlb")
        # --- 5A: scores.T + exp + denominator (whole-rank K.T loads) ---
        with tc.tile_pool(name="p5w", bufs=3) as p5w, \
             tc.tile_pool(name="p5ps", bufs=3, space="PSUM") as p5ps, \
             tc.tile_pool(name="p5lps", bufs=1, space="PSUM") as p5lps:
            l_ps = p5lps.tile([1, S], f32)
            for r in range(NCORES):
                kt_r = p5w.tile([P, FC, S], bf16, tag="ktr")
                kt_src = kt_in[:] if nocoll else kt_out[r]
                kt_ap = kt_src.rearrange("(p c s) -> p c s", p=P, s=S)
                # two half-loads: a single DMA gets limited SDMA-engine
                # parallelism, two in flight doubles effective bandwidth
                nc.sync.dma_start(out=kt_r[:, 0:FC // 2, :],
                                  in_=kt_ap[:, 0:FC // 2, :])
                nc.sync.dma_start(out=kt_r[:, FC // 2:, :],
                                  in_=kt_ap[:, FC // 2:, :])
                for cl in range(4):
                    jc = r * 4 + cl
                    jw = slice(cl * P, (cl + 1) * P)
                    s_ps = p5ps.tile([P, S], f32, tag="sps")
                    for ch in range(FC):
                        mm(s_ps, kt_r[:, ch, jw], qT[:, ch, :],
                           start=(ch == 0), stop=(ch == FC - 1))
                    nc.scalar.activation(pt[:, jc, :], s_ps, AF.Exp,
                                         bias=0.0, scale=1.0)
            # denominator after all score chains: the l-matmuls then never
            # block the PE queue on a pending exp
            for jc in range(NJ):
                mm(l_ps, ones_col, pt[:, jc, :],
                   start=(jc == 0), stop=(jc == NJ - 1))
            l_row = p_rl.tile([1, S], f32, tag="lrow")
            nc.vector.reciprocal(l_row, l_ps)
            nc.sync.dma_start(out=row_bounce[0:1, :], in_=l_row)
            nc.sync.dma_start(out=rl_b, in_=row_bounce[0:1, :].partition_broadcast(P))
        # fold 1/l into the gate while 5B's matmuls run: zatp = (f/l) * zat
        for ot in range(FC):
            nc.vector.tensor_mul(fT[:, ot, :], fT[:, ot, :], rl_b)

        # --- 5B: Zat.T (whole-rank V loads) ---
        with tc.tile_pool(name="p5v", bufs=3) as p5v, \
             tc.tile_pool(name="pvps", bufs=1, space="PSUM") as pvps:
            zat_ps = []
            for i in range(FC):
                zp = pvps.tile([P, S], f32, tag=f"zat{i}", name=f"zat{i}")
                zat_ps.append(zp)
            for r in range(NCORES):
                v_r = p5v.tile([P, 4, D], bf16, tag="vr")
                v_src = v_in[:] if nocoll else v_out[r]
                v_ap = v_src.rearrange("(t p o) -> p t o", p=P, o=D)
                nc.sync.dma_start(out=v_r[:, 0:2, :], in_=v_ap[:, 0:2, :])
                nc.sync.dma_start(out=v_r[:, 2:4, :], in_=v_ap[:, 2:4, :])
                for t in range(4):
                    jc = r * 4 + t
                    for ot in range(FC):
                        mm(zat_ps[ot], v_r[:, t, ot * P:(ot + 1) * P],
                           pt[:, jc, :],
                           start=(jc == 0), stop=(jc == NJ - 1))
            # zatp = (f/l) * zat  (1/l was folded into fT during the matmuls)
            for ot in range(FC):
                nc.vector.tensor_mul(zatp[:, ot, :], zat_ps[ot], fT[:, ot, :])

    _emit_head(nc, tc, AF, bf16, f32, mm, w_in, wfin, out, bsb, bias_ap,
               remap, zatp, i_row, fin_rem)


def _emit_head(nc, tc, AF, bf16, f32, mm, w_in, wfin, out, bsb, bias_ap,
               remap, zatp, i_row, fin_rem):
    # ---------------- Phase 6: output head ----------------
    # p_hat = sigmoid(i*(Ztp@Wf) + (1-i)*(remap@Wf)); remap@Wf was phase 4.
    with tc.tile_pool(name="p6w", bufs=2) as p6w, \
         tc.tile_pool(name="p6ps", bufs=3, space="PSUM") as p6ps, \
         tc.tile_pool(name="p6t", bufs=2) as p6t, \
         tc.tile_pool(name="p6fps", bufs=1, space="PSUM") as p6fps:
        wfin_sb = p6t.tile([P, FC], bf16, tag="wfin")
        nc.sync.dma_start(out=wfin_sb, in_=wfin.ap())
        fin_ps = p6fps.tile([1, S], f32)
        ztps = []
        for half in range(2):
            w_t = p6w.tile([P, FC, 4 * P], bf16, tag="w")
            nc.sync.dma_start(out=w_t, in_=w_in["wzat"].ap()[half])
            for sub in range(4):
                ot = half * 4 + sub
                ow = slice(sub * P, (sub + 1) * P)
                ps = p6ps.tile([P, S], f32, tag="ps")
                for ch in range(FC):
                    mm(ps, w_t[:, ch, ow], zatp[:, ch, :],
                       start=(ch == 0), stop=(ch == FC - 1))
                t_sum = p6t.tile([P, S], f32, tag="tsum")
                nc.vector.tensor_add(t_sum, ps, remap[:, ot, :])
                ztp = p6t.tile([P, S], bf16, tag=f"ztp{ot}", name=f"ztp{ot}")
                nc.scalar.activation(ztp, t_sum, AF.Tanh,
                                     bias=bias_ap(8, ot), scale=1.0)
                ztps.append(ztp)
        # fin matmuls after all tanh tiles: no per-tile PE stall on ACT
        for ot in range(FC):
            mm(fin_ps, wfin_sb[:, ot:ot + 1], ztps[ot],
               start=(ot == 0), stop=(ot == FC - 1))
        # blend the two head branches on the [1, S] rows
        d_r = p6t.tile([1, S], f32, tag="dr")
        nc.vector.tensor_sub(d_r, fin_ps, fin_rem)
        m_r = p6t.tile([1, S], f32, tag="mr")
        nc.vector.tensor_mul(m_r, d_r, i_row)
        s_r = p6t.tile([1, S], f32, tag="sr")
        nc.vector.tensor_add(s_r, m_r, fin_rem)
        phat = p6t.tile([1, S], f32, tag="phat")
        nc.scalar.activation(phat, s_r, AF.Sigmoid, bias=0.0, scale=1.0)
        nc.sync.dma_start(out=out.ap().rearrange("s o -> o s"), in_=phat)


def _prep_host_inputs(inputs):
    """Transpose/pack weights into SBUF layouts + build per-core shards."""
    import concourse.mybir as mybir
    bf16 = mybir.dt.np(mybir.dt.bfloat16)

    R = np.ascontiguousarray(inputs["R"], dtype=np.float32)
    RT_ext = np.concatenate(
        [np.zeros((D, 1), np.float32), np.ascontiguousarray(R.T)], axis=1)

    def pack_p1(w):  # W [D, 2D] -> W.T [2D, D] -> [ot, p, k(2FC), o(P)]
        wt = np.ascontiguousarray(np.asarray(w, np.float32).T)
        return np.ascontiguousarray(
            wt.reshape(2 * FC, P, FC, P).transpose(2, 1, 0, 3)).astype(bf16)

    def pack_proj(w):  # W [D, D] -> W.T [D, D] -> [half, p, c(FC), o(4P)]
        wt = np.ascontiguousarray(np.asarray(w, np.float32).T)
        return np.ascontiguousarray(
            wt.reshape(FC, P, 2, 4 * P).transpose(2, 1, 0, 3)).astype(bf16)

    def pack_col(w):  # W [1, D] -> [p, c(FC)]
        wt = np.asarray(w, np.float32).reshape(FC, P).T
        return np.ascontiguousarray(wt).astype(bf16)

    w = {
        "wa": pack_p1(inputs["W_alpha"]), "wd": pack_p1(inputs["W_delta"]),
        "wz": pack_proj(inputs["W_z"]), "wq": pack_proj(inputs["W_q"]),
        "wk": pack_proj(inputs["W_k"]), "wv": pack_proj(inputs["W_v"]),
        "wema": pack_proj(inputs["W_EMA"]), "wf": pack_proj(inputs["W_f"]),
        "wzat": pack_proj(inputs["W_z_at"]),
        "wi": pack_col(inputs["W_i"]), "wfin": pack_col(inputs["W_final"]),
    }

    brows = np.zeros((10, D), np.float32)
    brows[0] = inputs["b_alpha"]
    brows[1] = inputs["b_delta"]
    brows[2] = inputs["b_z"]
    brows[3] = inputs["b_q"] * ATT_SCALE
    brows[4] = inputs["b_k"]
    brows[5] = inputs["b_v"]
    brows[6] = inputs["b_EMA"]
    brows[7] = inputs["b_f"]
    brows[8] = inputs["b_z_at"]
    brows[9, 0] = np.float32(inputs["b_i"][0])
    biases = np.ascontiguousarray(
        brows.reshape(10, FC, P).transpose(2, 0, 1))  # [p, 10, FC]
    bvrow = np.ascontiguousarray(brows[5:6])  # [1, D]

    in_maps = []
    for c in range(NCORES):
        rt_c = RT_ext[:, c * S:c * S + S + 1]  # [D, S+1]
        rt_p = np.ascontiguousarray(
            rt_c.reshape(FC, P, S + 1).transpose(1, 0, 2)).astype(bf16)
        m = {"rt": rt_p, "biases": biases, "bvrow": bvrow}
        m.update(w)
        in_maps.append(m)
    return in_maps


def kernel(**inputs):
    from concourse.bass_utils import run_bass_kernel_spmd

    if "nc" not in _CACHE:
        _CACHE["nc"] = _build_bass()
    nc = _CACHE["nc"]
    in_maps = _prep_host_inputs(inputs)
    res = run_bass_kernel_spmd(nc, in_maps, core_ids=list(range(NCORES)))
    outs = [res.results[c]["out"] for c in range(NCORES)]
    return np.concatenate(outs, axis=0).astype(np.float32)
